# revision 19
# baseline (speedup 1.0000x reference)
"""Trainium2 Bass kernel: 1-layer transformer block w/ ALiBi bidirectional attention.

Sharding: data-parallel over batch (B=8) across 8 NeuronCores; zero collectives.

Per-core dataflow (S=1024, D=512, H=8, HD=64, FFN=2048), bf16 matmuls / fp32 PSUM:
  - Host supplies x pre-transposed (xT [d, s]); activations kept natural [s, d]
    for LayerNorm, transposed via PE (identity matmul) where matmuls need it.
  - ALiBi factorization: bias(s,t) = +-slope*(t - s) splits into a per-s term
    (folded into an augmented K=65 row of the q operand; its bf16 rounding is
    constant per softmax column so it cancels exactly in the normalization)
    and a per-t term (the per-partition ACT bias of the fused exp; scores are
    computed transposed: [t partitions, s free]).
  - Each head is half-masked (-1e9) => only the triangular half of the S x S
    score tiles is computed. Diagonal 128x128 tiles are masked by elementwise
    multiply with a 0/1 triangle.
  - q/k are projected per head ([64, 512] PSUM out) so no partition-shifting
    DMAs are needed to split heads.
  - softmax denominator r[s] comes free as an extra output row of the
    probs@V matmul (ones column appended to V); probs@V batches 4 s-chunks
    per PSUM bank; 1/r fused into the PSUM->attn_nat copy.
  - Attention head loop is software-pipelined: scores/exp run two heads
    ahead of probs@V so the PE never drains (avoids HAM re-throttle).
  - LN scale/bias of all three LNs folded into the following weight matrices
    host-side (exact algebra); LN stats batched: one Rsqrt per LN.
"""

import sys

import ml_dtypes
import numpy as np

sys.path.insert(0, "/opt/trn_rl_repo")

import concourse.bass as bass  # noqa: E402,F401
from concourse import bacc  # noqa: E402
import concourse.tile as tile  # noqa: E402
from concourse import mybir  # noqa: E402
from concourse.bass_utils import run_bass_kernel_spmd  # noqa: E402

F32 = mybir.dt.float32
BF16 = mybir.dt.bfloat16
NPBF16 = ml_dtypes.bfloat16
AF = mybir.ActivationFunctionType
OP = mybir.AluOpType

P = 128
B = 8
S = 1024
D = 512
H = 8
HD = 64
FFN = 4 * D
SM = S // P  # 8 sequence chunks
DK = D // P  # 4 feature chunks
FK = FFN // P  # 16 ffn chunks
EPS = 1e-5
N_CORES = 8


def _slopes():
    half = H // 2
    base = 24.0 ** (1.0 / half)
    return (1.0 / base ** np.arange(1, half + 1)).astype(np.float64)


def _fwd(h):
    return h < H // 2


# per (head, j) score-tile geometry for the transposed scores [t=j*128+p, s]
def _s_range(h, j):
    if _fwd(h):  # keep t <= s : s-chunks j..7
        return j * P, S - j * P
    else:  # keep t >= s : s-chunks 0..j
        return 0, (j + 1) * P


def _eoff(h, j):
    off = 0
    for jj in range(j):
        off += _s_range(h, jj)[1]
    return off


def _ewidth(h):
    return _eoff(h, SM - 1) + _s_range(h, SM - 1)[1]  # = 4608


def build_nc(gelu_mode="gelu"):
    nc = bacc.Bacc("TRN2", target_bir_lowering=False, debug=False)

    def din(name, shape, dt=F32):
        return nc.dram_tensor(name, list(shape), dt, kind="ExternalInput").ap()

    d = {}
    d["x"] = din("x", (D, S), BF16)  # pre-transposed host-side
    d["w_in"] = din("w_in", (D, D), BF16)
    d["b_in"] = din("b_in", (D,))
    d["wq"] = din("wq", (D, D), BF16)
    d["wk"] = din("wk", (D, D), BF16)
    d["wv"] = din("wv", (D, D), BF16)
    d["wo"] = din("wo", (D, D), BF16)
    d["bo"] = din("bo", (D,))
    d["w1"] = din("w1", (D, FFN), BF16)
    d["w2"] = din("w2", (FFN, D), BF16)
    d["b2"] = din("b2", (D,))
    d["w_out"] = din("w_out", (D, D), BF16)
    d["b_out"] = din("b_out", (D,))
    d["bqh"] = din("bqh", (HD, H))
    d["bkh"] = din("bkh", (HD, H))
    d["b1c"] = din("b1c", (P, FK))
    d["bv"] = din("bv", (D,))
    d["qrow"] = din("qrow", (H, S), BF16)
    d["tb"] = din("tb", (P, H * SM))
    d["maskf"] = din("maskf", (P, P), BF16)
    d["maskb"] = din("maskb", (P, P), BF16)
    d["ident"] = din("ident", (P, P), BF16)
    d["out"] = nc.dram_tensor("out", [S, D], F32, kind="ExternalOutput").ap()

    with tile.TileContext(nc) as tc:
        _emit(nc, tc, d, gelu_mode)
    nc.compile()
    return nc


def _emit(nc, tc, d, gelu_mode):
    pool = tc.alloc_tile_pool

    pc = pool(name="consts", bufs=1)
    ph = pool(name="resid", bufs=2)  # tag "h": h1, attn_nat, h2, h3 rotate
    phT = pool(name="transposed", bufs=2)  # tag "hT": xT,hn1T,attnT2,hn2T,hn3T
    psm = pool(name="smalls", bufs=4)
    phn = pool(name="hn_nat", bufs=2)
    pg = pool(name="gelu", bufs=3)
    posb = pool(name="outsb", bufs=3)
    pasb = pool(name="attnTsb", bufs=4)

    ps_mm = pool(name="ps_mm", bufs=2, space="PSUM")
    ps_acc = pool(name="ps_acc", bufs=4, space="PSUM")
    ps_tr = pool(name="ps_tr", bufs=2, space="PSUM")

    # ---- DMAs in consumption order: x, w_in first (critical path) ----
    xT = phT.tile([P, DK, S], BF16, tag="hT")
    x_view = d["x"].rearrange("(c p) n -> p c n", p=P)
    nc.sync.dma_start(out=xT[:, :, 0:512], in_=x_view[:, :, 0:512])
    nc.sync.dma_start(out=xT[:, :, 512:1024], in_=x_view[:, :, 512:1024])

    pwqkv = pool(name="wqkv", bufs=1)
    win_sb = pwqkv.tile([P, DK, D], BF16, tag="w_in")
    nc.sync.dma_start(out=win_sb, in_=d["w_in"].rearrange("(c p) n -> p c n", p=P))

    # small consts next (cheap)
    ident = pc.tile([P, P], BF16, tag="ident")
    nc.sync.dma_start(out=ident, in_=d["ident"])
    maskf = pc.tile([P, P], BF16, tag="maskf")
    nc.sync.dma_start(out=maskf, in_=d["maskf"])
    maskb = pc.tile([P, P], BF16, tag="maskb")
    nc.sync.dma_start(out=maskb, in_=d["maskb"])
    tb = pc.tile([P, H * SM], F32, tag="tb")
    nc.sync.dma_start(out=tb, in_=d["tb"])
    bqh = pc.tile([HD, H], F32, tag="bqh")
    nc.sync.dma_start(out=bqh, in_=d["bqh"])
    bkh = pc.tile([HD, H], F32, tag="bkh")
    nc.sync.dma_start(out=bkh, in_=d["bkh"])
    b1c = pc.tile([P, FK], F32, tag="b1c")
    nc.sync.dma_start(out=b1c, in_=d["b1c"])
    b1cs = pc.tile([P, FK], F32, tag="b1cs")
    nc.vector.tensor_scalar(b1cs, b1c, scalar1=1.702, scalar2=None, op0=OP.mult)

    def bcast(name):
        t = pc.tile([P, D], F32, tag=name + "B")
        nc.gpsimd.dma_start(out=t, in_=d[name].partition_broadcast(P))
        return t

    epsc = pc.tile([P, 1], F32, tag="epsc")
    nc.gpsimd.memset(epsc, EPS)

    binB = bcast("b_in")
    bvB = bcast("bv")
    boB = bcast("bo")
    b2B = bcast("b2")
    boutB = bcast("b_out")

    # remaining weights stream in behind the first-stage ones
    wv_sb = pwqkv.tile([P, DK, D], BF16, tag="wv")
    nc.sync.dma_start(out=wv_sb, in_=d["wv"].rearrange("(c p) n -> p c n", p=P))
    wq_sb = pwqkv.tile([P, DK, D], BF16, tag="wq")
    nc.sync.dma_start(out=wq_sb, in_=d["wq"].rearrange("(c p) n -> p c n", p=P))
    wk_sb = pwqkv.tile([P, DK, D], BF16, tag="wk")
    nc.sync.dma_start(out=wk_sb, in_=d["wk"].rearrange("(c p) n -> p c n", p=P))
    pwo = pool(name="wo_pool", bufs=1)
    wo_sb = pwo.tile([P, DK, D], BF16, tag="wo")
    nc.sync.dma_start(out=wo_sb, in_=d["wo"].rearrange("(c p) n -> p c n", p=P))
    pwbig = pool(name="wbig", bufs=1)
    w1_sb = pwbig.tile([P, DK, FFN], BF16, tag="w1")
    nc.sync.dma_start(out=w1_sb, in_=d["w1"].rearrange("(c p) n -> p c n", p=P))
    w2_sb = pwbig.tile([P, FK, D], BF16, tag="w2")
    nc.sync.dma_start(out=w2_sb, in_=d["w2"].rearrange("(c p) n -> p c n", p=P))
    wout_sb = pwbig.tile([P, DK, D], BF16, tag="w_out")
    nc.sync.dma_start(out=wout_sb, in_=d["w_out"].rearrange("(c p) n -> p c n", p=P))

    pqk = pool(name="qkheads", bufs=1)
    pva = pool(name="vaug", bufs=1)
    pexp = pool(name="expT", bufs=3)

    def transpose_to(dst, src, eng=0):
        # src [128,128] SBUF -> dst [128,128] (SBUF dest via PSUM bounce)
        t = ps_tr.tile([P, P], BF16, tag="tr")
        nc.tensor.transpose(t, src, ident)
        if eng == 0:
            nc.vector.tensor_copy(dst, t)
        else:
            nc.scalar.copy(dst, t)

    # h1 = x @ w_in + b_in    (natural)
    h1 = ph.tile([P, SM, D], BF16, tag="h")
    for m in range(SM):
        ps = ps_mm.tile([P, D], F32, tag="mm")
        for dk in range(DK):
            nc.tensor.matmul(
                ps,
                xT[:, dk, m * P : (m + 1) * P],
                win_sb[:, dk, :],
                start=(dk == 0),
                stop=(dk == DK - 1),
            )
        nc.vector.tensor_tensor(out=h1[:, m, :], in0=ps, in1=binB, op=OP.add)

    def ln_to_T(src, dstT):
        # batched LayerNorm of [P, SM, D] src into transposed dstT [P, DK, S]
        stats8 = psm.tile([P, SM, 6], F32, tag="st")
        mv8 = psm.tile([P, SM, 2], F32, tag="mv")
        for m in range(SM):
            nc.vector.bn_stats(stats8[:, m, :], src[:, m, :])
            nc.vector.bn_aggr(mv8[:, m, :], stats8[:, m, :])
        sq8 = psm.tile([P, SM], F32, tag="sq")
        nc.scalar.activation(sq8, mv8[:, :, 1], AF.Sqrt, bias=epsc)
        rstd8 = psm.tile([P, SM], F32, tag="rstd")
        nc.vector.reciprocal(rstd8, sq8)
        negmr8 = psm.tile([P, SM], F32, tag="negmr")
        nc.vector.tensor_tensor(out=negmr8, in0=mv8[:, :, 0], in1=rstd8, op=OP.mult)
        nc.vector.tensor_scalar(
            negmr8, negmr8, scalar1=-1.0, scalar2=None, op0=OP.mult
        )
        for m in range(SM):
            hn = phn.tile([P, D], BF16, tag="hn")
            nc.scalar.activation(
                hn,
                src[:, m, :],
                AF.Identity,
                bias=negmr8[:, m : m + 1],
                scale=rstd8[:, m : m + 1],
            )
            for dk in range(DK):
                transpose_to(
                    dstT[:, dk, m * P : (m + 1) * P],
                    hn[:, dk * P : (dk + 1) * P],
                    eng=dk % 2,
                )

    # hn1T = LN1(h1) transposed [d, s]
    hn1T = phT.tile([P, DK, S], BF16, tag="hT")
    ln_to_T(h1, hn1T)

    # ---- v projection, natural layout, augmented with ones column ----
    v_aug = pva.tile([P, SM, H, 65], BF16, tag="vaug")
    for t in range(SM):
        psv = ps_mm.tile([P, D], F32, tag="mm", name="psv")
        for dk in range(DK):
            nc.tensor.matmul(
                psv,
                hn1T[:, dk, t * P : (t + 1) * P],
                wv_sb[:, dk, :],
                start=(dk == 0),
                stop=(dk == DK - 1),
            )
        nc.vector.tensor_tensor(
            out=v_aug[:, t, :, 0:64],
            in0=psv.rearrange("p (h e) -> p h e", h=H),
            in1=bvB.rearrange("p (h e) -> p h e", h=H),
            op=OP.add,
        )
        nc.gpsimd.memset(v_aug[:, t, :, 64:65], 1.0)

    # ---- q/k projections, per head: [64, 512] PSUM, no partition moves ----
    qTa = {}
    kTa = {}
    for h in range(H):
        qTa[h] = pqk.tile([65, S], BF16, tag=f"qTa{h}", name=f"qTa{h}")
        nc.sync.dma_start(out=qTa[h][64:65, :], in_=d["qrow"][h : h + 1, :])
        kTa[h] = pqk.tile([65, S], BF16, tag=f"kTa{h}", name=f"kTa{h}")
        nc.gpsimd.memset(kTa[h][64:65, :], 1.0)
    for half in range(2):
        for h in range(H):
            for w_sb, bc, dst in ((wq_sb, bqh, qTa), (wk_sb, bkh, kTa)):
                psq = ps_mm.tile([P, D], F32, tag="mm", name="psq")
                for dk in range(DK):
                    nc.tensor.matmul(
                        psq[0:HD, :],
                        w_sb[:, dk, h * HD : (h + 1) * HD],
                        hn1T[:, dk, half * 512 : (half + 1) * 512],
                        start=(dk == 0),
                        stop=(dk == DK - 1),
                    )
                nc.scalar.activation(
                    dst[h][0:HD, half * 512 : (half + 1) * 512],
                    psq[0:HD, :],
                    AF.Identity,
                    bias=bc[:, h : h + 1],
                )

    # ---- attention: software-pipelined over heads ----
    attn_nat = ph.tile([P, SM, D], BF16, tag="h", name="attn_nat")
    expTs = {}

    def stage_scores(h):
        # scores + fused exp + diagonal mask for head h -> expTs[h]
        expT = pexp.tile([P, _ewidth(h)], BF16, tag="expT", name=f"expT{h}")
        expTs[h] = expT
        for j in range(SM):
            s0, w = _s_range(h, j)
            eo = _eoff(h, j)
            off = 0
            while off < w:
                pw = min(512, w - off)
                pss = ps_mm.tile([P, pw], F32, tag="mm", name="pss")
                nc.tensor.matmul(
                    pss,
                    kTa[h][:, j * P : (j + 1) * P],
                    qTa[h][:, s0 + off : s0 + off + pw],
                    start=True,
                    stop=True,
                )
                nc.scalar.activation(
                    expT[:, eo + off : eo + off + pw],
                    pss,
                    AF.Exp,
                    bias=tb[:, h * SM + j : h * SM + j + 1],
                    scale=0.125,
                )
                off += pw
            # mask the diagonal 128x128 block (keep t<=s fwd / t>=s bwd)
            dg = eo if _fwd(h) else eo + j * P
            msk = maskf if _fwd(h) else maskb
            nc.vector.tensor_tensor(
                out=expT[:, dg : dg + P],
                in0=expT[:, dg : dg + P],
                in1=msk,
                op=OP.mult,
            )

    pvs = {}

    def stage_pv(h):
        # probs @ V (unnormalized) + denominator row, 4 s-chunks per PSUM bank
        expT = expTs[h]
        for half in range(2):
            pv4 = ps_acc.tile([65, 4, P], F32, tag="acc", name=f"pv4_{h}_{half}")
            for mm in range(4):
                m = half * 4 + mm
                js = list(range(0, m + 1)) if _fwd(h) else list(range(m, SM))
                for i, j in enumerate(js):
                    s0, _w = _s_range(h, j)
                    col = _eoff(h, j) + (m * P - s0)
                    nc.tensor.matmul(
                        pv4[:, mm, :],
                        v_aug[:, j, h, :],
                        expT[:, col : col + P],
                        start=(i == 0),
                        stop=(i == len(js) - 1),
                    )
            asb4 = pasb.tile([65, 4, P], BF16, tag="asb")
            nc.vector.tensor_copy(asb4, pv4)
            pvs[(h, half)] = asb4

    def stage_tail(h):
        # transpose back to natural, fuse 1/r into the PSUM->SBUF copy
        for half in range(2):
            asb4 = pvs.pop((h, half))
            trp4 = ps_tr.tile([P, 4, 66], BF16, tag="tr", name=f"trp4_{h}_{half}")
            for mm in range(4):
                nc.tensor.transpose(
                    trp4[:, mm, 0:65], asb4[:, mm, :], ident[0:65, 0:65]
                )
            rden = psm.tile([P, 4, 1], F32, tag="rden")
            nc.vector.tensor_copy(rden, trp4[:, :, 64:65])
            rinv4 = psm.tile([P, 4, 1], F32, tag="rinv4")
            nc.vector.reciprocal(rinv4, rden)
            for mm in range(4):
                m = half * 4 + mm
                nc.vector.tensor_scalar(
                    attn_nat[:, m, h * HD : (h + 1) * HD],
                    trp4[:, mm, 0:64],
                    scalar1=rinv4[:, mm, :],
                    scalar2=None,
                    op0=OP.mult,
                )

    stage_scores(0)
    stage_scores(1)
    stage_pv(0)
    for h in range(H):
        if h + 2 < H:
            stage_scores(h + 2)
        if h + 1 < H:
            stage_pv(h + 1)
        stage_tail(h)

    # attn transposed for the output projection
    attnT2 = phT.tile([P, DK, S], BF16, tag="hT")
    for m in range(SM):
        for dk in range(DK):
            transpose_to(
                attnT2[:, dk, m * P : (m + 1) * P],
                attn_nat[:, m, dk * P : (dk + 1) * P],
                eng=dk % 2,
            )

    # h2 = h1 + attn @ wo + bo
    h2 = ph.tile([P, SM, D], BF16, tag="h")
    for m in range(SM):
        ps = ps_mm.tile([P, D], F32, tag="mm", name="pswo")
        for dk in range(DK):
            nc.tensor.matmul(
                ps,
                attnT2[:, dk, m * P : (m + 1) * P],
                wo_sb[:, dk, :],
                start=(dk == 0),
                stop=(dk == DK - 1),
            )
        nc.vector.tensor_tensor(out=h2[:, m, :], in0=ps, in1=boB, op=OP.add)
        nc.vector.tensor_tensor(
            out=h2[:, m, :], in0=h2[:, m, :], in1=h1[:, m, :], op=OP.add
        )

    # hn2T = LN2(h2) transposed
    hn2T = phT.tile([P, DK, S], BF16, tag="hT")
    ln_to_T(h2, hn2T)

    # ---- FFN: h3 = h2 + gelu(hn2 @ w1 + b1) @ w2 + b2 ----
    h3 = ph.tile([P, SM, D], BF16, tag="h")
    for half in range(2):
        accs = []
        for mm in range(4):
            accs.append(ps_acc.tile([P, D], F32, tag="acc", name=f"ff2ps{mm}"))
        for kc in range(FK):
            ps1 = ps_mm.tile([P, 512], F32, tag="mm", name="ff1ps")
            for dk in range(DK):
                nc.tensor.matmul(
                    ps1,
                    w1_sb[:, dk, kc * P : (kc + 1) * P],
                    hn2T[:, dk, half * 512 : (half + 1) * 512],
                    start=(dk == 0),
                    stop=(dk == DK - 1),
                )
            gt = pg.tile([P, 512], BF16, tag="gt")
            if gelu_mode == "gelu":
                nc.scalar.activation(gt, ps1, AF.Gelu, bias=b1c[:, kc : kc + 1])
            else:  # CoreSim lacks Gelu: x*sigmoid(1.702x) stand-in
                sg = pg.tile([P, 512], BF16, tag="sg")
                nc.scalar.activation(
                    sg, ps1, AF.Sigmoid, bias=b1cs[:, kc : kc + 1], scale=1.702
                )
                xb = pg.tile([P, 512], BF16, tag="xb")
                nc.vector.tensor_scalar(
                    xb, ps1, scalar1=b1c[:, kc : kc + 1], scalar2=None, op0=OP.add
                )
                nc.vector.tensor_tensor(out=gt, in0=sg, in1=xb, op=OP.mult)
            for mm in range(4):
                nc.tensor.matmul(
                    accs[mm],
                    gt[:, mm * P : (mm + 1) * P],
                    w2_sb[:, kc, :],
                    start=(kc == 0),
                    stop=(kc == FK - 1),
                )
        for mm in range(4):
            m = half * 4 + mm
            nc.vector.tensor_tensor(out=h3[:, m, :], in0=accs[mm], in1=b2B, op=OP.add)
            nc.vector.tensor_tensor(
                out=h3[:, m, :], in0=h3[:, m, :], in1=h2[:, m, :], op=OP.add
            )

    # ---- final LN + output projection ----
    hn3T = phT.tile([P, DK, S], BF16, tag="hT")
    ln_to_T(h3, hn3T)

    out_view = d["out"].rearrange("(c p) n -> p c n", p=P)
    for m in range(SM):
        ps = ps_mm.tile([P, D], F32, tag="mm", name="psout")
        for dk in range(DK):
            nc.tensor.matmul(
                ps,
                hn3T[:, dk, m * P : (m + 1) * P],
                wout_sb[:, dk, :],
                start=(dk == 0),
                stop=(dk == DK - 1),
            )
        osb = posb.tile([P, D], F32, tag="osb")
        nc.vector.tensor_tensor(out=osb, in0=ps, in1=boutB, op=OP.add)
        nc.sync.dma_start(out=out_view[:, m, :], in_=osb)

    for p_ in (pexp, pva, pqk, pwbig, pwo, pwqkv, ps_tr, ps_acc, ps_mm, pasb,
               posb, pg, phn, psm, phT, ph, pc):
        p_.release()


def host_prep(inputs):
    """Fold LN affine params into weights; build ALiBi helper tensors."""
    f = lambda k: np.asarray(inputs[k], dtype=np.float64)
    ln1_s, ln1_b = f("ln1_s"), f("ln1_b")
    ln2_s, ln2_b = f("ln2_s"), f("ln2_b")
    lnf_s, lnf_b = f("lnf_s"), f("lnf_b")
    wq, bq = f("wq"), f("bq")
    wk, bk = f("wk"), f("bk")
    wv, bv = f("wv"), f("bv")
    w1, b1 = f("w1"), f("b1")
    w_out, b_out = f("w_out"), f("b_out")

    wq_f = (ln1_s[:, None] * wq).astype(np.float32)
    bq_f = (bq + ln1_b @ wq).astype(np.float32)
    wk_f = (ln1_s[:, None] * wk).astype(np.float32)
    bk_f = (bk + ln1_b @ wk).astype(np.float32)
    wv_f = (ln1_s[:, None] * wv).astype(np.float32)
    bv_f = (bv + ln1_b @ wv).astype(np.float32)
    w1_f = (ln2_s[:, None] * w1).astype(np.float32)
    b1_f = (b1 + ln2_b @ w1).astype(np.float32)
    wout_f = (lnf_s[:, None] * w_out).astype(np.float32)
    bout_f = (b_out + lnf_b @ w_out).astype(np.float32)

    sl = _slopes()
    qrow = np.zeros((H, S), np.float32)
    tb = np.zeros((P, H * SM), np.float32)
    s_idx = np.arange(S, dtype=np.float64)
    p_idx = np.arange(P, dtype=np.float64)
    for h in range(H):
        sgn = -1.0 if h < H // 2 else 1.0  # sign of the per-s row term
        qrow[h] = (sgn * 8.0 * sl[h % 4] * s_idx).astype(np.float32)
        for j in range(SM):
            tb[:, h * SM + j] = (-sgn * sl[h % 4] * (j * P + p_idx)).astype(
                np.float32
            )
    maskf = np.triu(np.ones((P, P), np.float32))  # keep t <= s (p <= c)
    maskb = np.tril(np.ones((P, P), np.float32))  # keep t >= s (p >= c)

    common = {
        "w_in": np.asarray(inputs["w_in"], np.float32).astype(NPBF16),
        "b_in": np.asarray(inputs["b_in"], np.float32),
        "wq": wq_f.astype(NPBF16),
        "wk": wk_f.astype(NPBF16),
        "wv": wv_f.astype(NPBF16),
        "wo": np.asarray(inputs["wo"], np.float32).astype(NPBF16),
        "bo": np.asarray(inputs["bo"], np.float32),
        "w1": w1_f.astype(NPBF16),
        "w2": np.asarray(inputs["w2"], np.float32).astype(NPBF16),
        "b2": np.asarray(inputs["b2"], np.float32),
        "w_out": wout_f.astype(NPBF16),
        "b_out": bout_f,
        "bqh": np.ascontiguousarray(bq_f.reshape(H, HD).T),
        "bkh": np.ascontiguousarray(bk_f.reshape(H, HD).T),
        "b1c": np.ascontiguousarray(b1_f.reshape(FK, P).T),
        "bv": bv_f,
        "qrow": qrow.astype(NPBF16),
        "tb": tb,
        "maskf": maskf.astype(NPBF16),
        "maskb": maskb.astype(NPBF16),
        "ident": np.eye(P, dtype=NPBF16),
    }
    return common


_NC_CACHE = {}


def get_nc(gelu_mode="gelu"):
    if gelu_mode not in _NC_CACHE:
        _NC_CACHE[gelu_mode] = build_nc(gelu_mode)
    return _NC_CACHE[gelu_mode]


def run(inputs, trace=False):
    common = host_prep(inputs)
    x = np.asarray(inputs["x"], np.float32)
    in_maps = [
        dict(common, x=np.ascontiguousarray(x[i].T).astype(NPBF16))
        for i in range(N_CORES)
    ]
    nc = get_nc()
    res = run_bass_kernel_spmd(
        nc, in_maps, core_ids=list(range(N_CORES)), trace=trace
    )
    out = np.stack([res.results[i]["out"] for i in range(N_CORES)])
    return out.astype(np.float32), res


def kernel(**inputs):
    out, _ = run(inputs, trace=False)
    return out


# revision 21
# speedup vs baseline: 1.0559x; 1.0559x over previous
"""Trainium2 Bass kernel: 1-layer transformer block w/ ALiBi bidirectional attention.

Sharding: data-parallel over batch (B=8) across 8 NeuronCores; zero collectives.

Per-core dataflow (S=1024, D=512, H=8, HD=64, FFN=2048), bf16 matmuls / fp32 PSUM:
  - Host supplies x pre-transposed (xT [d, s]); activations kept natural [s, d]
    for LayerNorm, transposed via PE (identity matmul) where matmuls need it.
  - ALiBi factorization: bias(s,t) = +-slope*(t - s) splits into a per-s term
    (folded into an augmented K=65 row of the q operand; its bf16 rounding is
    constant per softmax column so it cancels exactly in the normalization)
    and a per-t term (the per-partition ACT bias of the fused exp; scores are
    computed transposed: [t partitions, s free]).
  - Each head is half-masked (-1e9) => only the triangular half of the S x S
    score tiles is computed. Diagonal 128x128 tiles are masked by elementwise
    multiply with a 0/1 triangle.
  - q/k are projected per head ([64, 512] PSUM out) so no partition-shifting
    DMAs are needed to split heads.
  - softmax denominator r[s] comes free as an extra output row of the
    probs@V matmul (ones column appended to V); probs@V batches 4 s-chunks
    per PSUM bank; 1/r fused into the PSUM->attn_nat copy.
  - Attention head loop is software-pipelined: scores/exp run two heads
    ahead of probs@V so the PE never drains (avoids HAM re-throttle).
  - LN scale/bias of all three LNs folded into the following weight matrices
    host-side (exact algebra); LN stats batched: one Rsqrt per LN.
"""

import sys

import ml_dtypes
import numpy as np

sys.path.insert(0, "/opt/trn_rl_repo")

import concourse.bass as bass  # noqa: E402,F401
from concourse import bacc  # noqa: E402
import concourse.tile as tile  # noqa: E402
from concourse import mybir  # noqa: E402
from concourse.bass_utils import run_bass_kernel_spmd  # noqa: E402

F32 = mybir.dt.float32
BF16 = mybir.dt.bfloat16
NPBF16 = ml_dtypes.bfloat16
AF = mybir.ActivationFunctionType
OP = mybir.AluOpType

P = 128
B = 8
S = 1024
D = 512
H = 8
HD = 64
FFN = 4 * D
SM = S // P  # 8 sequence chunks
DK = D // P  # 4 feature chunks
FK = FFN // P  # 16 ffn chunks
EPS = 1e-5
N_CORES = 8


def _slopes():
    half = H // 2
    base = 24.0 ** (1.0 / half)
    return (1.0 / base ** np.arange(1, half + 1)).astype(np.float64)


def _fwd(h):
    return h < H // 2


# per (head, j) score-tile geometry for the transposed scores [t=j*128+p, s]
def _s_range(h, j):
    if _fwd(h):  # keep t <= s : s-chunks j..7
        return j * P, S - j * P
    else:  # keep t >= s : s-chunks 0..j
        return 0, (j + 1) * P


def _eoff(h, j):
    off = 0
    for jj in range(j):
        off += _s_range(h, jj)[1]
    return off


def _ewidth(h):
    return _eoff(h, SM - 1) + _s_range(h, SM - 1)[1]  # = 4608


def build_nc(gelu_mode="gelu"):
    nc = bacc.Bacc("TRN2", target_bir_lowering=False, debug=False)

    def din(name, shape, dt=F32):
        return nc.dram_tensor(name, list(shape), dt, kind="ExternalInput").ap()

    d = {}
    d["x"] = din("x", (D, S), BF16)  # pre-transposed host-side
    d["w_in"] = din("w_in", (D, D), BF16)
    d["b_in"] = din("b_in", (D,))
    d["wq"] = din("wq", (D, D), BF16)
    d["wk"] = din("wk", (D, D), BF16)
    d["wv"] = din("wv", (D, D), BF16)
    d["wo"] = din("wo", (D, D), BF16)
    d["bo"] = din("bo", (D,))
    d["w1"] = din("w1", (D, FFN), BF16)
    d["w2"] = din("w2", (FFN, D), BF16)
    d["b2"] = din("b2", (D,))
    d["w_out"] = din("w_out", (D, D), BF16)
    d["b_out"] = din("b_out", (D,))
    d["bqh"] = din("bqh", (HD, H))
    d["bkh"] = din("bkh", (HD, H))
    d["b1c"] = din("b1c", (P, FK))
    d["bv"] = din("bv", (D,))
    d["qrow"] = din("qrow", (H, S), BF16)
    d["tb"] = din("tb", (P, H * SM))
    d["maskf"] = din("maskf", (P, P), BF16)
    d["maskb"] = din("maskb", (P, P), BF16)
    d["ident"] = din("ident", (P, P), BF16)
    d["out"] = nc.dram_tensor("out", [S, D], F32, kind="ExternalOutput").ap()

    with tile.TileContext(nc) as tc:
        _emit(nc, tc, d, gelu_mode)
    nc.compile()
    return nc


def _emit(nc, tc, d, gelu_mode):
    pool = tc.alloc_tile_pool

    pc = pool(name="consts", bufs=1)
    ph = pool(name="resid", bufs=2)  # tag "h": h1, attn_nat, h2, h3 rotate
    phT = pool(name="transposed", bufs=2)  # tag "hT": xT,hn1T,attnT2,hn2T,hn3T
    psm = pool(name="smalls", bufs=4)
    phn = pool(name="hn_nat", bufs=2)
    pg = pool(name="gelu", bufs=3)
    posb = pool(name="outsb", bufs=3)
    pasb = pool(name="attnTsb", bufs=4)

    ps_mm = pool(name="ps_mm", bufs=2, space="PSUM")
    ps_acc = pool(name="ps_acc", bufs=4, space="PSUM")
    ps_tr = pool(name="ps_tr", bufs=2, space="PSUM")

    # ---- DMAs in consumption order: x, w_in first (critical path) ----
    xT = phT.tile([P, DK, S], BF16, tag="hT")
    x_view = d["x"].rearrange("(c p) n -> p c n", p=P)
    nc.sync.dma_start(out=xT[:, :, 0:512], in_=x_view[:, :, 0:512])
    nc.sync.dma_start(out=xT[:, :, 512:1024], in_=x_view[:, :, 512:1024])

    pwqkv = pool(name="wqkv", bufs=1)
    win_sb = pwqkv.tile([P, DK, D], BF16, tag="w_in")
    nc.sync.dma_start(out=win_sb, in_=d["w_in"].rearrange("(c p) n -> p c n", p=P))

    # small consts next (cheap)
    ident = pc.tile([P, P], BF16, tag="ident")
    nc.sync.dma_start(out=ident, in_=d["ident"])
    maskf = pc.tile([P, P], BF16, tag="maskf")
    nc.sync.dma_start(out=maskf, in_=d["maskf"])
    maskb = pc.tile([P, P], BF16, tag="maskb")
    nc.sync.dma_start(out=maskb, in_=d["maskb"])
    tb = pc.tile([P, H * SM], F32, tag="tb")
    nc.sync.dma_start(out=tb, in_=d["tb"])
    bqh = pc.tile([HD, H], F32, tag="bqh")
    nc.sync.dma_start(out=bqh, in_=d["bqh"])
    bkh = pc.tile([HD, H], F32, tag="bkh")
    nc.sync.dma_start(out=bkh, in_=d["bkh"])
    b1c = pc.tile([P, FK], F32, tag="b1c")
    nc.sync.dma_start(out=b1c, in_=d["b1c"])
    b1cs = pc.tile([P, FK], F32, tag="b1cs")
    nc.vector.tensor_scalar(b1cs, b1c, scalar1=1.702, scalar2=None, op0=OP.mult)

    def bcast(name):
        t = pc.tile([P, D], F32, tag=name + "B")
        nc.gpsimd.dma_start(out=t, in_=d[name].partition_broadcast(P))
        return t

    epsc = pc.tile([P, 1], F32, tag="epsc")
    nc.gpsimd.memset(epsc, EPS)

    binB = bcast("b_in")
    bvB = bcast("bv")
    boB = bcast("bo")
    b2B = bcast("b2")
    boutB = bcast("b_out")

    # remaining weights stream in behind the first-stage ones
    wv_sb = pwqkv.tile([P, DK, D], BF16, tag="wv")
    nc.sync.dma_start(out=wv_sb, in_=d["wv"].rearrange("(c p) n -> p c n", p=P))
    wq_sb = pwqkv.tile([P, DK, D], BF16, tag="wq")
    nc.sync.dma_start(out=wq_sb, in_=d["wq"].rearrange("(c p) n -> p c n", p=P))
    wk_sb = pwqkv.tile([P, DK, D], BF16, tag="wk")
    nc.sync.dma_start(out=wk_sb, in_=d["wk"].rearrange("(c p) n -> p c n", p=P))
    pwo = pool(name="wo_pool", bufs=1)
    wo_sb = pwo.tile([P, DK, D], BF16, tag="wo")
    nc.sync.dma_start(out=wo_sb, in_=d["wo"].rearrange("(c p) n -> p c n", p=P))
    pwbig = pool(name="wbig", bufs=1)
    w1_sb = pwbig.tile([P, DK, FFN], BF16, tag="w1")
    nc.sync.dma_start(out=w1_sb, in_=d["w1"].rearrange("(c p) n -> p c n", p=P))
    w2_sb = pwbig.tile([P, FK, D], BF16, tag="w2")
    nc.sync.dma_start(out=w2_sb, in_=d["w2"].rearrange("(c p) n -> p c n", p=P))
    wout_sb = pwbig.tile([P, DK, D], BF16, tag="w_out")
    nc.sync.dma_start(out=wout_sb, in_=d["w_out"].rearrange("(c p) n -> p c n", p=P))

    pqk = pool(name="qkheads", bufs=1)
    pva = pool(name="vaug", bufs=1)
    pexp = pool(name="expT", bufs=3)

    def transpose_to(dst, src, eng=0):
        # src [128,128] SBUF -> dst [128,128] (SBUF dest via PSUM bounce)
        t = ps_tr.tile([P, P], BF16, tag="tr")
        nc.tensor.transpose(t, src, ident)
        if eng == 0:
            nc.vector.tensor_copy(dst, t)
        else:
            nc.scalar.copy(dst, t)

    # h1 = x @ w_in + b_in    (natural)
    h1 = ph.tile([P, SM, D], BF16, tag="h")
    for m in range(SM):
        ps = ps_mm.tile([P, D], F32, tag="mm")
        for dk in range(DK):
            nc.tensor.matmul(
                ps,
                xT[:, dk, m * P : (m + 1) * P],
                win_sb[:, dk, :],
                start=(dk == 0),
                stop=(dk == DK - 1),
            )
        nc.vector.tensor_tensor(out=h1[:, m, :], in0=ps, in1=binB, op=OP.add)

    def ln_to_T(src, dstT):
        # per-chunk LayerNorm of [P, SM, D] src into transposed dstT [P, DK, S]
        for m in range(SM):
            stats = psm.tile([P, 6], F32, tag="st")
            nc.vector.bn_stats(stats, src[:, m, :])
            mv = psm.tile([P, 2], F32, tag="mv")
            nc.vector.bn_aggr(mv, stats)
            sq = psm.tile([P, 1], F32, tag="sq")
            nc.scalar.activation(sq, mv[:, 1:2], AF.Sqrt, bias=epsc)
            rstd = psm.tile([P, 1], F32, tag="rstd")
            nc.vector.reciprocal(rstd, sq)
            negmr = psm.tile([P, 1], F32, tag="negmr")
            nc.vector.tensor_scalar(
                negmr, mv[:, 0:1], scalar1=rstd, scalar2=-1.0,
                op0=OP.mult, op1=OP.mult,
            )
            hn = phn.tile([P, D], BF16, tag="hn")
            nc.vector.tensor_scalar(
                hn, src[:, m, :], scalar1=rstd, scalar2=negmr,
                op0=OP.mult, op1=OP.add,
            )
            for dk in range(DK):
                transpose_to(
                    dstT[:, dk, m * P : (m + 1) * P],
                    hn[:, dk * P : (dk + 1) * P],
                    eng=dk % 2,
                )

    # hn1T = LN1(h1) transposed [d, s]
    hn1T = phT.tile([P, DK, S], BF16, tag="hT")
    ln_to_T(h1, hn1T)

    # ---- v projection, natural layout, augmented with ones column ----
    v_aug = pva.tile([P, SM, H, 65], BF16, tag="vaug")
    for t in range(SM):
        psv = ps_mm.tile([P, D], F32, tag="mm", name="psv")
        for dk in range(DK):
            nc.tensor.matmul(
                psv,
                hn1T[:, dk, t * P : (t + 1) * P],
                wv_sb[:, dk, :],
                start=(dk == 0),
                stop=(dk == DK - 1),
            )
        nc.vector.tensor_tensor(
            out=v_aug[:, t, :, 0:64],
            in0=psv.rearrange("p (h e) -> p h e", h=H),
            in1=bvB.rearrange("p (h e) -> p h e", h=H),
            op=OP.add,
        )
        nc.gpsimd.memset(v_aug[:, t, :, 64:65], 1.0)

    # ---- q/k projections, per head: [64, 512] PSUM, no partition moves ----
    qTa = {}
    kTa = {}
    for h in range(H):
        qTa[h] = pqk.tile([65, S], BF16, tag=f"qTa{h}", name=f"qTa{h}")
        nc.scalar.dma_start(out=qTa[h][64:65, :], in_=d["qrow"][h : h + 1, :])
        kTa[h] = pqk.tile([65, S], BF16, tag=f"kTa{h}", name=f"kTa{h}")
        nc.gpsimd.memset(kTa[h][64:65, :], 1.0)
    for h in range(H):
        for w_sb, bc, dst in ((wq_sb, bqh, qTa), (wk_sb, bkh, kTa)):
            for half in range(2):
                psq = ps_mm.tile([P, D], F32, tag="mm", name="psq")
                for dk in range(DK):
                    nc.tensor.matmul(
                        psq[0:HD, :],
                        w_sb[:, dk, h * HD : (h + 1) * HD],
                        hn1T[:, dk, half * 512 : (half + 1) * 512],
                        start=(dk == 0),
                        stop=(dk == DK - 1),
                    )
                nc.vector.tensor_scalar(
                    dst[h][0:HD, half * 512 : (half + 1) * 512],
                    psq[0:HD, :],
                    scalar1=bc[:, h : h + 1],
                    scalar2=None,
                    op0=OP.add,
                )

    # ---- attention: software-pipelined over heads ----
    attn_nat = ph.tile([P, SM, D], BF16, tag="h", name="attn_nat")
    expTs = {}

    def stage_scores(h):
        # scores + fused exp + diagonal mask for head h -> expTs[h]
        expT = pexp.tile([P, _ewidth(h)], BF16, tag="expT", name=f"expT{h}")
        expTs[h] = expT
        for j in range(SM):
            s0, w = _s_range(h, j)
            eo = _eoff(h, j)
            off = 0
            while off < w:
                pw = min(512, w - off)
                pss = ps_mm.tile([P, pw], F32, tag="mm", name="pss")
                nc.tensor.matmul(
                    pss,
                    kTa[h][:, j * P : (j + 1) * P],
                    qTa[h][:, s0 + off : s0 + off + pw],
                    start=True,
                    stop=True,
                )
                nc.scalar.activation(
                    expT[:, eo + off : eo + off + pw],
                    pss,
                    AF.Exp,
                    bias=tb[:, h * SM + j : h * SM + j + 1],
                    scale=0.125,
                )
                off += pw
            # mask the diagonal 128x128 block (keep t<=s fwd / t>=s bwd)
            dg = eo if _fwd(h) else eo + j * P
            msk = maskf if _fwd(h) else maskb
            nc.vector.tensor_tensor(
                out=expT[:, dg : dg + P],
                in0=expT[:, dg : dg + P],
                in1=msk,
                op=OP.mult,
            )

    pvs = {}

    def stage_pv(h):
        # probs @ V (unnormalized) + denominator row, 4 s-chunks per PSUM bank
        expT = expTs[h]
        for half in range(2):
            pv4 = ps_acc.tile([65, 4, P], F32, tag="acc", name=f"pv4_{h}_{half}")
            for mm in range(4):
                m = half * 4 + mm
                js = list(range(0, m + 1)) if _fwd(h) else list(range(m, SM))
                for i, j in enumerate(js):
                    s0, _w = _s_range(h, j)
                    col = _eoff(h, j) + (m * P - s0)
                    nc.tensor.matmul(
                        pv4[:, mm, :],
                        v_aug[:, j, h, :],
                        expT[:, col : col + P],
                        start=(i == 0),
                        stop=(i == len(js) - 1),
                    )
            asb4 = pasb.tile([65, 4, P], BF16, tag="asb")
            nc.vector.tensor_copy(asb4, pv4)
            pvs[(h, half)] = asb4

    def stage_tail(h):
        # transpose back to natural, fuse 1/r into the PSUM->SBUF copy
        for half in range(2):
            asb4 = pvs.pop((h, half))
            trp4 = ps_tr.tile([P, 4, 66], BF16, tag="tr", name=f"trp4_{h}_{half}")
            for mm in range(4):
                nc.tensor.transpose(
                    trp4[:, mm, 0:65], asb4[:, mm, :], ident[0:65, 0:65]
                )
            rden = psm.tile([P, 4, 1], F32, tag="rden")
            nc.vector.tensor_copy(rden, trp4[:, :, 64:65])
            rinv4 = psm.tile([P, 4, 1], F32, tag="rinv4")
            nc.vector.reciprocal(rinv4, rden)
            for mm in range(4):
                m = half * 4 + mm
                nc.vector.tensor_scalar(
                    attn_nat[:, m, h * HD : (h + 1) * HD],
                    trp4[:, mm, 0:64],
                    scalar1=rinv4[:, mm, :],
                    scalar2=None,
                    op0=OP.mult,
                )

    stage_scores(0)
    stage_scores(1)
    stage_pv(0)
    for h in range(H):
        if h + 2 < H:
            stage_scores(h + 2)
        if h + 1 < H:
            stage_pv(h + 1)
        stage_tail(h)

    # attn transposed for the output projection
    attnT2 = phT.tile([P, DK, S], BF16, tag="hT")
    for m in range(SM):
        for dk in range(DK):
            transpose_to(
                attnT2[:, dk, m * P : (m + 1) * P],
                attn_nat[:, m, dk * P : (dk + 1) * P],
                eng=dk % 2,
            )

    # h2 = h1 + attn @ wo + bo
    h2 = ph.tile([P, SM, D], BF16, tag="h")
    for m in range(SM):
        ps = ps_mm.tile([P, D], F32, tag="mm", name="pswo")
        for dk in range(DK):
            nc.tensor.matmul(
                ps,
                attnT2[:, dk, m * P : (m + 1) * P],
                wo_sb[:, dk, :],
                start=(dk == 0),
                stop=(dk == DK - 1),
            )
        nc.vector.tensor_tensor(out=h2[:, m, :], in0=ps, in1=boB, op=OP.add)
        nc.vector.tensor_tensor(
            out=h2[:, m, :], in0=h2[:, m, :], in1=h1[:, m, :], op=OP.add
        )

    # hn2T = LN2(h2) transposed
    hn2T = phT.tile([P, DK, S], BF16, tag="hT")
    ln_to_T(h2, hn2T)

    # ---- FFN: h3 = h2 + gelu(hn2 @ w1 + b1) @ w2 + b2 ----
    h3 = ph.tile([P, SM, D], BF16, tag="h")
    for half in range(2):
        accs = []
        for mm in range(4):
            accs.append(ps_acc.tile([P, D], F32, tag="acc", name=f"ff2ps{mm}"))
        for kc in range(FK):
            ps1 = ps_mm.tile([P, 512], F32, tag="mm", name="ff1ps")
            for dk in range(DK):
                nc.tensor.matmul(
                    ps1,
                    w1_sb[:, dk, kc * P : (kc + 1) * P],
                    hn2T[:, dk, half * 512 : (half + 1) * 512],
                    start=(dk == 0),
                    stop=(dk == DK - 1),
                )
            gt = pg.tile([P, 512], BF16, tag="gt")
            if gelu_mode == "gelu":
                nc.scalar.activation(gt, ps1, AF.Gelu, bias=b1c[:, kc : kc + 1])
            else:  # CoreSim lacks Gelu: x*sigmoid(1.702x) stand-in
                sg = pg.tile([P, 512], BF16, tag="sg")
                nc.scalar.activation(
                    sg, ps1, AF.Sigmoid, bias=b1cs[:, kc : kc + 1], scale=1.702
                )
                xb = pg.tile([P, 512], BF16, tag="xb")
                nc.vector.tensor_scalar(
                    xb, ps1, scalar1=b1c[:, kc : kc + 1], scalar2=None, op0=OP.add
                )
                nc.vector.tensor_tensor(out=gt, in0=sg, in1=xb, op=OP.mult)
            for mm in range(4):
                nc.tensor.matmul(
                    accs[mm],
                    gt[:, mm * P : (mm + 1) * P],
                    w2_sb[:, kc, :],
                    start=(kc == 0),
                    stop=(kc == FK - 1),
                )
        for mm in range(4):
            m = half * 4 + mm
            nc.vector.tensor_tensor(out=h3[:, m, :], in0=accs[mm], in1=b2B, op=OP.add)
            nc.vector.tensor_tensor(
                out=h3[:, m, :], in0=h3[:, m, :], in1=h2[:, m, :], op=OP.add
            )

    # ---- final LN + output projection ----
    hn3T = phT.tile([P, DK, S], BF16, tag="hT")
    ln_to_T(h3, hn3T)

    out_view = d["out"].rearrange("(c p) n -> p c n", p=P)
    for m in range(SM):
        ps = ps_mm.tile([P, D], F32, tag="mm", name="psout")
        for dk in range(DK):
            nc.tensor.matmul(
                ps,
                hn3T[:, dk, m * P : (m + 1) * P],
                wout_sb[:, dk, :],
                start=(dk == 0),
                stop=(dk == DK - 1),
            )
        osb = posb.tile([P, D], F32, tag="osb")
        nc.vector.tensor_tensor(out=osb, in0=ps, in1=boutB, op=OP.add)
        eng = (nc.sync, nc.scalar, nc.gpsimd)[m % 3]
        eng.dma_start(out=out_view[:, m, :], in_=osb)

    for p_ in (pexp, pva, pqk, pwbig, pwo, pwqkv, ps_tr, ps_acc, ps_mm, pasb,
               posb, pg, phn, psm, phT, ph, pc):
        p_.release()


def host_prep(inputs):
    """Fold LN affine params into weights; build ALiBi helper tensors."""
    f = lambda k: np.asarray(inputs[k], dtype=np.float64)
    ln1_s, ln1_b = f("ln1_s"), f("ln1_b")
    ln2_s, ln2_b = f("ln2_s"), f("ln2_b")
    lnf_s, lnf_b = f("lnf_s"), f("lnf_b")
    wq, bq = f("wq"), f("bq")
    wk, bk = f("wk"), f("bk")
    wv, bv = f("wv"), f("bv")
    w1, b1 = f("w1"), f("b1")
    w_out, b_out = f("w_out"), f("b_out")

    wq_f = (ln1_s[:, None] * wq).astype(np.float32)
    bq_f = (bq + ln1_b @ wq).astype(np.float32)
    wk_f = (ln1_s[:, None] * wk).astype(np.float32)
    bk_f = (bk + ln1_b @ wk).astype(np.float32)
    wv_f = (ln1_s[:, None] * wv).astype(np.float32)
    bv_f = (bv + ln1_b @ wv).astype(np.float32)
    w1_f = (ln2_s[:, None] * w1).astype(np.float32)
    b1_f = (b1 + ln2_b @ w1).astype(np.float32)
    wout_f = (lnf_s[:, None] * w_out).astype(np.float32)
    bout_f = (b_out + lnf_b @ w_out).astype(np.float32)

    sl = _slopes()
    qrow = np.zeros((H, S), np.float32)
    tb = np.zeros((P, H * SM), np.float32)
    s_idx = np.arange(S, dtype=np.float64)
    p_idx = np.arange(P, dtype=np.float64)
    for h in range(H):
        sgn = -1.0 if h < H // 2 else 1.0  # sign of the per-s row term
        qrow[h] = (sgn * 8.0 * sl[h % 4] * s_idx).astype(np.float32)
        for j in range(SM):
            tb[:, h * SM + j] = (-sgn * sl[h % 4] * (j * P + p_idx)).astype(
                np.float32
            )
    maskf = np.triu(np.ones((P, P), np.float32))  # keep t <= s (p <= c)
    maskb = np.tril(np.ones((P, P), np.float32))  # keep t >= s (p >= c)

    common = {
        "w_in": np.asarray(inputs["w_in"], np.float32).astype(NPBF16),
        "b_in": np.asarray(inputs["b_in"], np.float32),
        "wq": wq_f.astype(NPBF16),
        "wk": wk_f.astype(NPBF16),
        "wv": wv_f.astype(NPBF16),
        "wo": np.asarray(inputs["wo"], np.float32).astype(NPBF16),
        "bo": np.asarray(inputs["bo"], np.float32),
        "w1": w1_f.astype(NPBF16),
        "w2": np.asarray(inputs["w2"], np.float32).astype(NPBF16),
        "b2": np.asarray(inputs["b2"], np.float32),
        "w_out": wout_f.astype(NPBF16),
        "b_out": bout_f,
        "bqh": np.ascontiguousarray(bq_f.reshape(H, HD).T),
        "bkh": np.ascontiguousarray(bk_f.reshape(H, HD).T),
        "b1c": np.ascontiguousarray(b1_f.reshape(FK, P).T),
        "bv": bv_f,
        "qrow": qrow.astype(NPBF16),
        "tb": tb,
        "maskf": maskf.astype(NPBF16),
        "maskb": maskb.astype(NPBF16),
        "ident": np.eye(P, dtype=NPBF16),
    }
    return common


_NC_CACHE = {}


def get_nc(gelu_mode="gelu"):
    if gelu_mode not in _NC_CACHE:
        _NC_CACHE[gelu_mode] = build_nc(gelu_mode)
    return _NC_CACHE[gelu_mode]


def run(inputs, trace=False):
    common = host_prep(inputs)
    x = np.asarray(inputs["x"], np.float32)
    in_maps = [
        dict(common, x=np.ascontiguousarray(x[i].T).astype(NPBF16))
        for i in range(N_CORES)
    ]
    nc = get_nc()
    res = run_bass_kernel_spmd(
        nc, in_maps, core_ids=list(range(N_CORES)), trace=trace
    )
    out = np.stack([res.results[i]["out"] for i in range(N_CORES)])
    return out.astype(np.float32), res


def kernel(**inputs):
    out, _ = run(inputs, trace=False)
    return out
